# revision 5
# baseline (speedup 1.0000x reference)
"""AugmentedLstm Trainium2 kernel (hidden-major, three-source gate PSUM).

Math (faithful to the reference):
    g_t = (x_t + h_{t-1}) @ W + 2b     (the module projects input and state
                                        with the SAME W_in)
    i,f,o,hw = sigmoid(g_*);  m = tanh(g_m);  nhw = sigmoid(-g_hw) = 1-hw
    c_t  = i*m + f*c_{t-1}
    e2   = (o*hw) * tanh(c_t);  p1 = nhw * px5;  px5 = x_t @ W6 + b6
    h_t  = e2 + p1;  output_t = mask(t < len) * h_t   (mask applied on host;
           dead rows compute garbage harmlessly - lengths sorted descending)

Sharding: data-parallel over batch, 16 rows/core, full local scan per core.

Layout: hidden-major [hidden-unit partitions, batch free].  Gate matmuls put
gates on PSUM partitions (out [128,16]) so each matmul streams only 16 moving
rows, h never needs a PE transpose, and the gate bias becomes one indicator
matmul.  g is accumulated from three moving sources - bias+x (early), p1@W
(mid-step), e2@W (critical tail) - into three PSUM tiles (if / m / o,hw,nhw)
so sigmoid(i,f) waits only on its own 32+1 matmuls.  The per-step critical
chain is sig(if) -> tanh(m) -> i*m -> +f*c -> tanh(c) -> e2 -> 32 matmuls.

Cost-model timeline: 1.256 ms for B=128,T=512,H=512 on 8 cores (baseline
6.103 ms); rel err 8.2e-3 vs the fp64 reference.
"""

import numpy as np
from contextlib import ExitStack

import concourse.bass as bass
import concourse.bacc as bacc
import concourse.tile as tile
import concourse.mybir as mybir

F32 = mybir.dt.float32
BF16 = mybir.dt.bfloat16
AF = mybir.ActivationFunctionType
ALU = mybir.AluOpType

B, T, H = 128, 512, 512
NCORES = 8
BSH = B // NCORES
KC = H // 128
NCH = 24                   # i0-3 f0-3 o0-3 hw0-3 nhw0-3 m0-3
SB = 8


def build_nc_v8(t_steps=T, bsh=BSH):
    nc = bacc.Bacc(
        "TRN2",
        target_bir_lowering=False,
        debug=False,
        enable_asserts=False,
        num_devices=NCORES,
    )
    nblk = t_steps // SB
    xT_d = nc.dram_tensor("xT", [nblk, 128, KC, SB, bsh], BF16, kind="ExternalInput")
    w5_d = nc.dram_tensor("w5", [128, KC, NCH, 128], BF16, kind="ExternalInput")
    w6_d = nc.dram_tensor("w6", [128, KC, KC, 128], BF16, kind="ExternalInput")
    b5T_d = nc.dram_tensor("b5T", [NCH, 128], BF16, kind="ExternalInput")
    ind5_d = nc.dram_tensor("ind5", [NCH, NCH * bsh], BF16, kind="ExternalInput")
    b6T_d = nc.dram_tensor("b6T", [KC, 128], BF16, kind="ExternalInput")
    ind6_d = nc.dram_tensor("ind6", [KC, KC * bsh], BF16, kind="ExternalInput")
    out_d = nc.dram_tensor("out", [nblk, 128, SB, KC, bsh], BF16, kind="ExternalOutput")

    with tile.TileContext(nc) as tc:
        with ExitStack() as ctx:
            const = ctx.enter_context(tc.tile_pool(name="const", bufs=1))
            xpool = ctx.enter_context(tc.tile_pool(name="xp", bufs=3))
            mcpool = ctx.enter_context(tc.tile_pool(name="mcp", bufs=2))
            spool = ctx.enter_context(tc.tile_pool(name="sp", bufs=2))
            pxpool = ctx.enter_context(tc.tile_pool(name="pxp", bufs=2))
            hpool = ctx.enter_context(tc.tile_pool(name="hp", bufs=2))
            gifp = ctx.enter_context(
                tc.tile_pool(name="gif", bufs=2, space=bass.MemorySpace.PSUM))
            gmp = ctx.enter_context(
                tc.tile_pool(name="gm", bufs=2, space=bass.MemorySpace.PSUM))
            grp = ctx.enter_context(
                tc.tile_pool(name="gr", bufs=2, space=bass.MemorySpace.PSUM))
            ppsum = ctx.enter_context(
                tc.tile_pool(name="pps", bufs=2, space=bass.MemorySpace.PSUM))

            w5sb = const.tile([128, KC, NCH, 128], BF16, tag="w5")
            nc.sync.dma_start(w5sb[:], w5_d[:])
            w6sb = const.tile([128, KC, KC, 128], BF16, tag="w6")
            nc.sync.dma_start(w6sb[:], w6_d[:])
            b5if = const.tile([8, 128], BF16, tag="b5if")
            nc.sync.dma_start(b5if[:], b5T_d[0:8])
            b5m = const.tile([4, 128], BF16, tag="b5m")
            nc.sync.dma_start(b5m[:], b5T_d[20:24])
            b5r = const.tile([12, 128], BF16, tag="b5r")
            nc.sync.dma_start(b5r[:], b5T_d[8:20])
            indif = const.tile([8, 8 * bsh], BF16, tag="indif")
            nc.sync.dma_start(indif[:], ind5_d[0:8, 0 : 8 * bsh])
            indm = const.tile([4, 4 * bsh], BF16, tag="indm")
            nc.sync.dma_start(indm[:], ind5_d[20:24, 20 * bsh : 24 * bsh])
            indr = const.tile([12, 12 * bsh], BF16, tag="indr")
            nc.sync.dma_start(indr[:], ind5_d[8:20, 8 * bsh : 20 * bsh])
            b6Tsb = const.tile([KC, 128], BF16, tag="b6T")
            nc.sync.dma_start(b6Tsb[:], b6T_d[:])
            ind6sb = const.tile([KC, KC * bsh], BF16, tag="ind6")
            nc.sync.dma_start(ind6sb[:], ind6_d[:])

            # m (bf16, ACT-written each step) and c (f32 state) tiles
            mtt = mcpool.tile([128, KC, bsh], BF16, tag="mt")
            ct = mcpool.tile([128, KC, bsh], F32, tag="ct")
            nc.vector.memset(ct[:], 0.0)

            def mm4(dst, ch, src_t, stop):
                for ck in range(KC):
                    nc.tensor.matmul(
                        dst, w5sb[:, ck, ch, :], src_t[:, ck, :],
                        start=False, stop=stop and (ck == KC - 1),
                        skip_group_check=True,
                    )

            def emit_gates_x(xt, s):
                """bias + x-part of g(t): allocs tiles, no recurrent deps."""
                gif = gifp.tile([128, 8, bsh], F32, tag="gif", name="gif")
                gm = gmp.tile([128, KC, bsh], F32, tag="gm", name="gm")
                gr = grp.tile([128, 12, bsh], F32, tag="gr", name="gr")
                xs = xt[:, :, s, :]
                nc.tensor.matmul(
                    gif[:].rearrange("p c b -> p (c b)"), b5if[:], indif[:],
                    start=True, stop=False, skip_group_check=True)
                for ch in range(8):
                    mm4(gif[:, ch, :], ch, xs, False)
                nc.tensor.matmul(
                    gm[:].rearrange("p c b -> p (c b)"), b5m[:], indm[:],
                    start=True, stop=False, skip_group_check=True)
                for ch in range(20, 24):
                    mm4(gm[:, ch - 20, :], ch, xs, False)
                nc.tensor.matmul(
                    gr[:].rearrange("p c b -> p (c b)"), b5r[:], indr[:],
                    start=True, stop=False, skip_group_check=True)
                for ch in range(8, 20):
                    mm4(gr[:, ch - 8, :], ch, xs, False)
                return gif, gm, gr

            def emit_gates_p1(g3, p1_t):
                gif, gm, gr = g3
                for ch in range(8):
                    mm4(gif[:, ch, :], ch, p1_t, False)

            def emit_gates_e2(g3, e2_t, h_t):
                gif, gm, gr = g3
                for ch in range(8):
                    mm4(gif[:, ch, :], ch, e2_t, True)
                for ch in range(20, 24):
                    mm4(gm[:, ch - 20, :], ch, h_t, True)
                for ch in range(8, 20):
                    mm4(gr[:, ch - 8, :], ch, h_t, True)

            def emit_px(xt, s):
                px = ppsum.tile([128, KC, bsh], F32, tag="px", name="px")
                nc.tensor.matmul(
                    px[:].rearrange("p c b -> p (c b)"),
                    b6Tsb[:], ind6sb[:],
                    start=True, stop=False, skip_group_check=True)
                for cc in range(KC):
                    for ck in range(KC):
                        nc.tensor.matmul(
                            px[:, cc, :], w6sb[:, ck, cc, :], xt[:, ck, s, :],
                            start=False, stop=(ck == KC - 1),
                            skip_group_check=True)
                return px

            nblk_ = nblk
            xtiles = {}
            for k in range(min(2, nblk_)):
                xtiles[k] = xpool.tile([128, KC, SB, bsh], BF16, tag="xt8",
                                       name=f"xt8_{k}")
                nc.sync.dma_start(xtiles[k][:], xT_d[k])

            # step 0: g(0) = x0 @ W + 2b only (h(-1) = 0); close the groups
            gcur = emit_gates_x(xtiles[0], 0)
            zt = const.tile([128, KC, bsh], BF16, tag="zt")
            nc.vector.memset(zt[:], 0.0)
            emit_gates_p1(gcur, zt)
            emit_gates_e2(gcur, zt, zt)
            px = emit_px(xtiles[0], 0)

            for blk in range(nblk_):
                xt8 = xtiles[blk]
                xt8_next = xtiles.get(blk + 1)
                if blk + 2 < nblk_:
                    xtiles[blk + 2] = xpool.tile(
                        [128, KC, SB, bsh], BF16, tag="xt8",
                        name=f"xt8_{blk + 2}")
                    nc.sync.dma_start(xtiles[blk + 2][:], xT_d[blk + 2])

                hstH = hpool.tile([128, SB, KC, bsh], BF16, tag="hstH")

                for s in range(SB):
                    t = blk * SB + s
                    gif, gm, gr = gcur

                    sigif = spool.tile([128, 8, bsh], BF16, tag="sigif")
                    nc.scalar.activation(sigif[:], gif[:], AF.Sigmoid)
                    mtt = mcpool.tile([128, KC, bsh], BF16, tag="mt")
                    nc.scalar.activation(mtt[:], gm[:], AF.Tanh)
                    sr = spool.tile([128, 12, bsh], BF16, tag="sr")
                    nc.scalar.activation(sr[:], gr[:], AF.Sigmoid)

                    # c path: fc first (needs only sig-f), im waits tanh-m;
                    # im/e2 all-bf16 SBUF for the DVE 2x mode, c stays f32
                    fc = spool.tile([128, KC, bsh], F32, tag="fc")
                    nc.vector.tensor_mul(fc[:], sigif[:, 4:8, :], ct[:])
                    im = spool.tile([128, KC, bsh], BF16, tag="im")
                    nc.vector.tensor_mul(im[:], sigif[:, 0:4, :], mtt[:])
                    cn = mcpool.tile([128, KC, bsh], F32, tag="ct")
                    nc.vector.tensor_add(cn[:], im[:], fc[:])
                    tc_ = spool.tile([128, KC, bsh], BF16, tag="tc")
                    nc.scalar.activation(tc_[:], cn[:], AF.Tanh)

                    # next-step tiles + x-part + px: x-only, fills PE idle
                    if t + 1 < t_steps:
                        if s + 1 < SB:
                            gnext = emit_gates_x(xt8, s + 1)
                            pxn = emit_px(xt8, s + 1)
                        else:
                            gnext = emit_gates_x(xt8_next, 0)
                            pxn = emit_px(xt8_next, 0)
                    else:
                        gnext = None
                        pxn = None

                    # p1 then its matmuls (run inside the tanh-c window)
                    p1 = pxpool.tile([128, KC, bsh], BF16, tag="p1")
                    nc.vector.tensor_mul(p1[:], sr[:, 8:12, :], px[:])
                    if gnext is not None:
                        emit_gates_p1(gnext, p1)
                    ohw = spool.tile([128, KC, bsh], BF16, tag="ohw")
                    nc.vector.tensor_mul(ohw[:], sr[:, 0:4, :], sr[:, 4:8, :])

                    # critical tail
                    e2 = spool.tile([128, KC, bsh], BF16, tag="e2")
                    nc.vector.tensor_mul(e2[:], ohw[:], tc_[:])
                    # h = e2 + p1 (off-path; feeds m/rest matmuls + store)
                    h = hstH[:, s]
                    nc.vector.scalar_tensor_tensor(
                        h, e2[:], 1.0, p1[:], op0=ALU.mult, op1=ALU.add)
                    if gnext is not None:
                        emit_gates_e2(gnext, e2, h)
                        gcur = gnext
                        px = pxn

                    ct = cn

                nc.sync.dma_start(out_d[blk], hstH[:])
                xtiles.pop(blk)

    nc.compile()
    return nc


def prep_inputs_v8(x, lengths, W_in, b_in, t_steps=T):
    iw = np.r_[0:512]
    fw = np.r_[512:1024]
    mw = np.r_[1024:1536]
    ow = np.r_[1536:2048]
    hww = np.r_[2048:2560]
    W5 = np.concatenate([
        W_in[:, iw], W_in[:, fw], W_in[:, ow], W_in[:, hww],
        -W_in[:, hww], W_in[:, mw]], axis=1).astype(np.float32)
    b5 = 2.0 * np.concatenate([
        b_in[iw], b_in[fw], b_in[ow], b_in[hww],
        -b_in[hww], b_in[mw]]).astype(np.float32)
    W6 = W_in[:, 2560:3072].astype(np.float32)
    b6 = b_in[2560:3072].astype(np.float32)

    def bf(a):
        import ml_dtypes
        return a.astype(ml_dtypes.bfloat16)

    w5r = np.ascontiguousarray(W5.reshape(KC, 128, NCH, 128).transpose(1, 0, 2, 3))
    w6r = np.ascontiguousarray(W6.reshape(KC, 128, KC, 128).transpose(1, 0, 2, 3))
    b5T = np.ascontiguousarray(b5.reshape(NCH, 128))
    ind5 = np.zeros((NCH, NCH, BSH), np.float32)
    for ch in range(NCH):
        ind5[ch, ch, :] = 1.0
    ind5 = ind5.reshape(NCH, NCH * BSH)
    b6T = np.ascontiguousarray(b6.reshape(KC, 128))
    ind6 = np.zeros((KC, KC, BSH), np.float32)
    for cc in range(KC):
        ind6[cc, cc, :] = 1.0
    ind6 = ind6.reshape(KC, KC * BSH)

    in_maps = []
    for j in range(NCORES):
        rows = slice(BSH * j, BSH * (j + 1))
        xc = x[rows, :t_steps]
        xT = np.ascontiguousarray(
            xc.reshape(BSH, t_steps // SB, SB, KC, 128).transpose(1, 4, 3, 2, 0))
        in_maps.append({
            "xT": bf(xT), "w5": bf(w5r), "w6": bf(w6r), "b5T": bf(b5T),
            "ind5": bf(ind5), "b6T": bf(b6T), "ind6": bf(ind6),
        })
    return in_maps


def postprocess_v8(results, lengths, t_steps=T):
    mask = (np.arange(t_steps)[None, :] < np.asarray(lengths)[:, None])
    out = np.empty((B, t_steps, H), np.float32)
    for j in range(NCORES):
        rows = slice(BSH * j, BSH * (j + 1))
        h = np.asarray(results[j]["out"], dtype=np.float32)
        out[rows] = h.transpose(4, 0, 2, 3, 1).reshape(BSH, t_steps, H)
    out *= mask[:, :, None]
    return out


VARIANT = 8


def build_nc(t_steps=T, bsh=BSH, variant=VARIANT):
    return build_nc_v8(t_steps)


def kernel(x, lengths, W_in, b_in):
    """Full-input entry point: shards batch over 8 cores, runs the Bass
    kernel SPMD, reassembles the full [B, T, H] float32 output."""
    from concourse.bass_utils import run_bass_kernel_spmd

    x = np.asarray(x, np.float32)
    lengths = np.asarray(lengths).astype(np.int64)
    W_in = np.asarray(W_in, np.float32)
    b_in = np.asarray(b_in, np.float32)

    nc = build_nc_v8(T)
    in_maps = prep_inputs_v8(x, lengths, W_in, b_in, t_steps=T)
    res = run_bass_kernel_spmd(nc, in_maps, list(range(NCORES)))
    if getattr(res, "exec_time_ns", None) is not None:
        print(f"HW exec time: {res.exec_time_ns} ns", flush=True)
    out = postprocess_v8(res.results, lengths, t_steps=T)
    return out.astype(np.float32)
